# revision 1
# baseline (speedup 1.0000x reference)
"""Trainium2 Bass kernel for an ODE-RNN encoder (z0 posterior).

Model: 128-step reversed-time GRU-like recurrence with an Euler ODE step on
the mean channel, then a final transform producing (mean_z0, std_z0).

Strategy: data-parallel over the subject (batch) dim across 8 NeuronCores,
weights replicated.  Everything runs on-chip in a transposed layout
([feature, batch], batch=256 on the free dim).  Key points:
- matmul operands and the recurrent state are bf16 (fp32 PSUM accumulate):
  fp32 matmuls on TRN2 lower to TWO half-speed PE passes, bf16 is
  single-pass with fast weight load.  Host-validated: ~5.5e-3 max rel
  error vs the fp32 reference (the matmul-input rounding dominates;
  bf16 state adds nothing measurable).
- all biases ride either a ones-row appended to the streamed x tile
  (layer-1 nets) or ACT per-partition bias vectors — zero per-step bias
  matmuls;  Yode = Ym + dt*(ode_out + b2) becomes one ACT op with
  per-partition scale=dt and bias=dt*b2 (host-precomputed per step);
- sigmoid(z) = 0.5 + 0.5*tanh(z/2) keeps every transcendental in the
  resident `exp_and_others` ACT table set (no per-step table switches);
- softplus(x) = log1p(exp(x)) via one Newton step on y: e^y = 1 + e^x,
  seeded with y0 = relu(x) + ln2*exp(-|x|) (~1.2e-3, under the bf16
  floor), algebraically folded into the gate blend so only two DVE ops
  trail the final exp;
- the update gate, the observation mask (broadcast via an all-ones K=64
  selector matmul) and the GRU convex combination fold into one factor
  G = 0.5*m*(1-tanh(zU/2)):  new = old + G*(cand - old); the (cand - old)
  part is accumulated in PSUM via a negative-identity matmul;
- TRN2 allows ONE sync wait per instruction; Bacc legalizes the rest, but
  K=1 dummy matmuls + accumulation-group ordering keep the PE free of
  multi-wait event-semaphore preambles in the steady state.
"""
import sys
import numpy as np
import ml_dtypes

for _p in ("/opt/trn_rl_repo", "/root/.axon_site/_ro/trn_rl_repo"):
    if _p not in sys.path:
        sys.path.append(_p)

N_SUBJ, N_TP, INPUT_DIM, LATENT, N_UNIT = 2048, 128, 64, 128, 256
HALF = INPUT_DIM // 2
N_CORES = 8
B = N_SUBJ // N_CORES          # 256 subjects per core (free dim)
L = LATENT
SP_ITERS = 1                   # softplus Newton refinements
LN2 = float(np.log(2.0))
BF = ml_dtypes.bfloat16

_CACHE = {}


# --------------------------------------------------------------------------
# Bass program
# --------------------------------------------------------------------------
def _build(n_tp, sp_iters):
    import concourse.mybir as mybir
    from concourse import bacc, tile

    F32 = mybir.dt.float32
    B16 = mybir.dt.bfloat16
    AF = mybir.ActivationFunctionType
    OP = mybir.AluOpType

    # Bacc (not plain Bass): its compile() legalizes the TRN2 one-sync-wait-
    # per-instruction limit (event-semaphore splitting, matmul-wait moves).
    nc = bacc.Bacc(None)

    # ---- DRAM I/O ----
    d_x = nc.dram_tensor("x_rev", [n_tp, INPUT_DIM, B], B16, kind="ExternalInput")

    bspec = {  # bf16 weights (matmul operands)
        "ug1_k0": [L, N_UNIT], "ug1_k1": [L, N_UNIT], "ug1_kx": [INPUT_DIM + 1, N_UNIT],
        "rg1_k0": [L, N_UNIT], "rg1_k1": [L, N_UNIT], "rg1_kx": [INPUT_DIM + 1, N_UNIT],
        "ns1_k0": [L, N_UNIT], "ns1_k1": [L, N_UNIT], "ns1_kx": [INPUT_DIM + 1, N_UNIT],
        "ode1_w": [L, N_UNIT],
        "ode2_k0": [128, L], "ode2_k1": [128, L],
        "ug2_k0": [128, L], "ug2_k1": [128, L],
        "rg2_k0": [128, L], "rg2_k1": [128, L],
        "ns2_k0": [128, 2 * L], "ns2_k1": [128, 2 * L], "ns2_bm16": [1, L],
        "neg_eye": [L, L],
        "tz1_k0": [L, N_UNIT], "tz1_k1": [L, N_UNIT], "tz1_b": [1, N_UNIT],
        "tz2_k0": [128, 2 * L], "tz2_k1": [128, 2 * L],
    }
    fspec = {  # fp32 per-partition columns (ACT bias/scale, STT scalar APs)
        "ode1_bc": [128, 2], "ug2_bc": [128, 1], "rg2_bc": [128, 1],
        "ns2_bm": [128, 1], "ns2_bs": [128, 1], "tz2_bm": [128, 1], "tz2_bs": [128, 1],
        "dt_b": [128, n_tp], "b2dt": [128, n_tp],
    }
    d_w = {k: nc.dram_tensor(k, v, B16, kind="ExternalInput") for k, v in bspec.items()}
    d_w.update({k: nc.dram_tensor(k, v, F32, kind="ExternalInput")
                for k, v in fspec.items()})

    d_om = nc.dram_tensor("out_m", [L, B], F32, kind="ExternalOutput")
    d_os = nc.dram_tensor("out_s", [L, B], F32, kind="ExternalOutput")

    with tile.TileContext(nc) as tc:
        with (
            tc.tile_pool(name="const", bufs=1) as cp,
            tc.tile_pool(name="work", bufs=3) as wp,
            tc.tile_pool(name="ps", bufs=1, space="PSUM") as pp,
        ):
            # ---- resident constants / weights ----
            w = {}
            for k, shp in bspec.items():
                w[k] = cp.tile(shp, B16, tag=k, name=k)
                nc.sync.dma_start(w[k][:], d_w[k][:])
            for k, shp in fspec.items():
                w[k] = cp.tile(shp, F32, tag=k, name=k)
                nc.sync.dma_start(w[k][:], d_w[k][:])
            ones_row = cp.tile([1, B], B16, tag="ones_row", name="ones_row")
            nc.vector.memset(ones_row[:], 1.0)
            # mask-channel selector: zeros over value rows, ones over mask rows
            msel = cp.tile([INPUT_DIM, 128], B16, tag="msel", name="msel")
            nc.vector.memset(msel[:HALF, :], 0.0)
            nc.vector.memset(msel[HALF:, :], 1.0)

            xbufs = []
            for j in range(3):
                xb = cp.tile([INPUT_DIM + 1, B], B16, tag=f"xb{j}", name=f"xb{j}")
                nc.vector.memset(xb[INPUT_DIM:, :], 1.0)
                xbufs.append(xb)

            # state lives in bf16 (host-validated: the bf16 matmul-input
            # rounding dominates; bf16 state adds nothing measurable) so no
            # fp32->bf16 casts sit on the recurrence
            ym = [cp.tile([L, B], B16, tag=f"ym{i}", name=f"ym{i}") for i in range(2)]
            ys = [cp.tile([L, B], B16, tag=f"ys{i}", name=f"ys{i}") for i in range(2)]
            nc.vector.memset(ym[0][:], 0.0)
            nc.vector.memset(ys[0][:], 0.0)

            mm = nc.tensor.matmul

            # Warm the PE's clock past every weight DMA with K=1 dummy
            # matmuls so steady-state matmuls only wait on one producer.
            scr = pp.tile([1, 16], F32, tag="scr", name="scr")
            for k in bspec:
                mm(scr[0:1, 0:1], w[k][0:1, 0:1], w[k][0:1, 1:2],
                   start=True, stop=True)
            # DVE/ACT read fp32 DMA-produced columns: warm those clocks too
            nf = len(fspec)
            warm_dv = cp.tile([1, 2 * nf], F32, tag="warm_dv", name="warm_dv")
            for j, k in enumerate(fspec):
                nc.vector.tensor_copy(warm_dv[0:1, j:j + 1], w[k][0:1, 0:1])
                nc.scalar.copy(warm_dv[0:1, nf + j:nf + j + 1], w[k][0:1, 0:1])

            # ---- the recurrence ----
            from concourse.tile_rust import add_dep_helper
            for t in range(n_tp):
                cur, nxt = t % 2, (t + 1) % 2
                xb = xbufs[t % 3]
                nc.sync.dma_start(xb[:INPUT_DIM, :], d_x[t])
                # absorb the x-DMA wait into a K=1 dummy
                mm(scr[0:1, 0:1], xb[0:1, 0:1], xb[0:1, 1:2], start=True, stop=True)

                # One start=True per PSUM bank per step (it clears the whole
                # bank's has_written bits); every other matmul accumulates or
                # first-touch-overwrites per element, so groups can interleave
                # freely.  x-only matmuls go first: they are ready before the
                # previous step's state tail finishes, keeping the PE fed.
                psA = pp.tile([128, 4 * B], F32, tag="psA", name="psA")
                psC = pp.tile([128, 2 * B], F32, tag="psC", name="psC")
                psF = pp.tile([128, 2 * B], F32, tag="psF", name="psF")
                # mask colsum broadcast opens the psF bank
                mm(psF[:, B:], msel[:], xb[:INPUT_DIM, :], start=True, stop=False)
                for gi, net in ((1, "rg1"), (0, "ug1")):
                    for m in range(2):
                        sl = psA[:, (2 * gi + m) * B:(2 * gi + m + 1) * B]
                        ms = slice(m * 128, (m + 1) * 128)
                        mm(sl, w[net + "_kx"][:, ms], xb[:],
                           start=(m == 0), stop=False)
                for m in range(2):
                    ms = slice(m * 128, (m + 1) * 128)
                    mm(psC[:, m * B:(m + 1) * B], w["ns1_kx"][:, ms], xb[:],
                       start=(m == 0), stop=False)

                # ODE hidden: tanh(ode_w1^T @ Ym + b1); split per m-half so
                # the ode2 k0 matmul starts as soon as half A is done
                psB = pp.tile([128, 2 * B], F32, tag="psB", name="psB")
                h_ode = wp.tile([128, 2 * B], B16, tag="h_ode", name="h_ode")
                for m in range(2):
                    sl = psB[:, m * B:(m + 1) * B]
                    ms = slice(m * 128, (m + 1) * 128)
                    mm(sl, w["ode1_w"][:, ms], ym[cur][:], start=(m == 0), stop=(m == 1))
                    nc.scalar.activation(h_ode[:, m * B:(m + 1) * B], sl, AF.Tanh,
                                         bias=w["ode1_bc"][:, m:m + 1])
                    mm(psF[:, 0:B], w[f"ode2_k{m}"][:], h_ode[:, m * B:(m + 1) * B],
                       start=False, stop=(m == 1))

                # T = dt*(ode_out + b2) via ACT scale/bias columns;
                # Yode = Ym + T (fp32) plus a bf16 copy for the PE
                t_ode = wp.tile([L, B], F32, tag="t_ode", name="t_ode")
                nc.scalar.activation(t_ode[:], psF[:, 0:B], AF.Identity,
                                     bias=w["b2dt"][:, t:t + 1],
                                     scale=w["dt_b"][:, t:t + 1])
                yode = wp.tile([L, B], B16, tag="yode", name="yode")
                nc.vector.tensor_tensor(yode[:], t_ode[:], ym[cur][:], op=OP.add)

                # gate layer 1 remaining k-tiles; yode (ready first) then ys,
                # per-group contiguous so each m-half completes as soon as its
                # last input lands and its tanh can fire
                for gi, net in ((1, "rg1"), (0, "ug1")):
                    for m in range(2):
                        sl = psA[:, (2 * gi + m) * B:(2 * gi + m + 1) * B]
                        ms = slice(m * 128, (m + 1) * 128)
                        mm(sl, w[net + "_k0"][:, ms], yode[:], start=False, stop=False)
                        mm(sl, w[net + "_k1"][:, ms], ys[cur][:], start=False,
                           stop=(m == 1))

                # ns1 base parts: W*((1+Tr).Y) = W*Y + W*(Tr.Y); the W*Y
                # halves accumulate here, before the reset gate exists
                for m in range(2):
                    sl = psC[:, m * B:(m + 1) * B]
                    ms = slice(m * 128, (m + 1) * 128)
                    mm(sl, w["ns1_k0"][:, ms], yode[:], start=False, stop=False)
                    mm(sl, w["ns1_k1"][:, ms], ys[cur][:], start=False, stop=False)

                # layer 2 per gate half; rg (reset gate) first: the critical
                # chain runs through R -> am2/as2 -> ns1, U is only needed at
                # the final blend
                h_g1 = wp.tile([128, 4 * B], B16, tag="h_g1", name="h_g1")
                psD = pp.tile([128, 2 * B], F32, tag="psD", name="psD")
                t_ur = wp.tile([128, 2 * B], B16, tag="t_ur", name="t_ur")
                for gi, net in ((1, "rg"), (0, "ug")):
                    hbase = 2 * gi * B
                    sl = psD[:, gi * B:(gi + 1) * B]
                    nc.scalar.activation(h_g1[:, hbase:hbase + B],
                                         psA[:, hbase:hbase + B], AF.Tanh)
                    mm(sl, w[net + "2_k0"][:], h_g1[:, hbase:hbase + B],
                       start=(gi == 1), stop=False)
                    nc.scalar.activation(h_g1[:, hbase + B:hbase + 2 * B],
                                         psA[:, hbase + B:hbase + 2 * B], AF.Tanh)
                    mm(sl, w[net + "2_k1"][:], h_g1[:, hbase + B:hbase + 2 * B],
                       start=False, stop=(gi == 0))
                    nc.scalar.activation(t_ur[:, gi * B:(gi + 1) * B], sl, AF.Tanh,
                                         bias=w[net + "2_bc"][:, 0:1], scale=0.5)

                # reset-gate correction products (ns1 k0/k1 pre-scaled 0.5,
                # so the early W*Y part + this W*(Tr.Y) part carry factor 2)
                am2 = wp.tile([L, B], B16, tag="am2", name="am2")
                nc.vector.tensor_tensor(am2[:], t_ur[:, B:], yode[:], op=OP.mult)
                as2 = wp.tile([L, B], B16, tag="as2", name="as2")
                nc.vector.tensor_tensor(as2[:], t_ur[:, B:], ys[cur][:], op=OP.mult)
                for m in range(2):
                    sl = psC[:, m * B:(m + 1) * B]
                    ms = slice(m * 128, (m + 1) * 128)
                    mm(sl, w["ns1_k0"][:, ms], am2[:], start=False, stop=False)
                    mm(sl, w["ns1_k1"][:, ms], as2[:], start=False, stop=(m == 1))

                # new-state layer 2: NM | NS pre-acts.  The NM half also
                # accumulates (+bm - Yode); nosync deps keep the bank's
                # start=True matmul first in the PE schedule.
                # NS half first: it feeds the softplus chain, which closes the
                # critical std-channel cycle; NM is only needed at the blend
                h_ns = wp.tile([128, 2 * B], B16, tag="h_ns", name="h_ns")
                psE = pp.tile([128, 2 * B], F32, tag="psE", name="psE")
                nc.scalar.activation(h_ns[:, 0:B], psC[:, 0:B], AF.Tanh)
                i_k0s = mm(psE[:, B:], w["ns2_k0"][:, 128:], h_ns[:, 0:B],
                           start=True, stop=False)
                i_k0m = mm(psE[:, 0:B], w["ns2_k0"][:, 0:128], h_ns[:, 0:B],
                           start=False, stop=False)
                i_bm = mm(psE[:, 0:B], w["ns2_bm16"][:], ones_row[:],
                          start=False, stop=False)
                i_ne = mm(psE[:, 0:B], w["neg_eye"][:], yode[:],
                          start=False, stop=False)
                # the bank's start=True matmul must execute first: bm/neg_eye
                # are ready earlier and would otherwise be scheduled ahead
                add_dep_helper(i_k0m.ins, i_k0s.ins, False, "bank-start order")
                add_dep_helper(i_bm.ins, i_k0s.ins, False, "bank-start order")
                add_dep_helper(i_ne.ins, i_bm.ins, False, "bank-start order")
                nc.scalar.activation(h_ns[:, B:], psC[:, B:], AF.Tanh)
                mm(psE[:, B:], w["ns2_k1"][:, 128:], h_ns[:, B:],
                   start=False, stop=False)
                mm(psE[:, 0:B], w["ns2_k1"][:, 0:128], h_ns[:, B:],
                   start=False, stop=True)

                # G = 0.5*m*(1 - T_u)
                t1 = wp.tile([L, B], F32, tag="t1", name="t1")
                nc.vector.tensor_scalar(t1[:], t_ur[:, 0:B], -0.5, 0.5,
                                        op0=OP.mult, op1=OP.add)
                g = wp.tile([L, B], F32, tag="g", name="g")
                nc.vector.scalar_tensor_tensor(
                    g[:], psF[:, B:], 0.0, t1[:], op0=OP.is_gt, op1=OP.mult)

                # mean channel: Ym' = Yode + G*(NM + bm - Yode)
                pm = wp.tile([L, B], F32, tag="pm", name="pm")
                nc.vector.tensor_tensor(pm[:], g[:], psE[:, 0:B], op=OP.mult)
                nc.vector.tensor_tensor(ym[nxt][:], yode[:], pm[:], op=OP.add)

                # std channel: softplus(x)=log1p(e^x) via Newton, then gate.
                # ACT order: Abs -> exp(-|x|) is the chain; E can run later
                # (only needed once the Newton product happens)
                xa = wp.tile([L, B], F32, tag="xa", name="xa")
                nc.scalar.activation(xa[:], psE[:, B:], AF.Abs,
                                     bias=w["ns2_bs"][:, 0:1])
                wx = wp.tile([L, B], B16, tag="wx", name="wx")
                nc.scalar.activation(wx[:], xa[:], AF.Exp, scale=-1.0)
                e_t = wp.tile([L, B], F32, tag="e_t", name="e_t")
                nc.scalar.activation(e_t[:], psE[:, B:], AF.Exp,
                                     bias=w["ns2_bs"][:, 0:1])
                rl = wp.tile([L, B], F32, tag="rl", name="rl")
                nc.vector.tensor_scalar(rl[:], psE[:, B:], w["ns2_bs"][:, 0:1],
                                        0.0, op0=OP.add, op1=OP.max)
                # One-iteration Newton, restructured so only two DVE ops
                # remain after the exp:  sp+1e-6 = y0 + c + A*u (c = -1+1e-6)
                #   Ys' = Ys + G*(sp+1e-6-Ys) = P + (G*A)*u
                #   P = Ys + G*(y0 + c - Ys)   (ready before the exp lands)
                a_t = wp.tile([L, B], F32, tag="a_t", name="a_t")
                nc.vector.tensor_scalar(a_t[:], e_t[:], 1.0, None, op0=OP.add)
                y_sp = wp.tile([L, B], F32, tag="ysp0", name="ysp0")
                nc.vector.scalar_tensor_tensor(
                    y_sp[:], wx[:], LN2, rl[:], op0=OP.mult, op1=OP.add)
                u_t = wp.tile([L, B], F32, tag="usp", name="usp")
                nc.scalar.activation(u_t[:], y_sp[:], AF.Exp, scale=-1.0)
                cc = float(np.float32(1e-6) - np.float32(1.0))
                h1 = wp.tile([L, B], F32, tag="h1", name="h1")
                nc.vector.scalar_tensor_tensor(
                    h1[:], y_sp[:], cc, ys[cur][:], op0=OP.add, op1=OP.subtract)
                p1 = wp.tile([L, B], F32, tag="p1", name="p1")
                nc.vector.tensor_tensor(p1[:], g[:], h1[:], op=OP.mult)
                p_ = wp.tile([L, B], F32, tag="p_", name="p_")
                nc.vector.tensor_tensor(p_[:], ys[cur][:], p1[:], op=OP.add)
                gt = wp.tile([L, B], F32, tag="gt", name="gt")
                nc.vector.tensor_tensor(gt[:], g[:], a_t[:], op=OP.mult)
                q_ = wp.tile([L, B], F32, tag="q_", name="q_")
                nc.vector.tensor_tensor(q_[:], gt[:], u_t[:], op=OP.mult)
                nc.vector.tensor_tensor(ys[nxt][:], p_[:], q_[:], op=OP.add)


            # ---- final transform ----
            fin = n_tp % 2
            psB = pp.tile([128, 2 * B], F32, tag="psB", name="psB")
            for m in range(2):
                sl = psB[:, m * B:(m + 1) * B]
                ms = slice(m * 128, (m + 1) * 128)
                mm(sl, w["tz1_b"][:, ms], ones_row[:], start=True, stop=False)
                mm(sl, w["tz1_k0"][:, ms], ym[fin][:], start=False, stop=False)
                mm(sl, w["tz1_k1"][:, ms], ys[fin][:], start=False, stop=True)
            h_tz = wp.tile([128, 2 * B], B16, tag="h_ode", name="h_tz")
            nc.scalar.activation(h_tz[:], psB[:], AF.Tanh)
            psE = pp.tile([128, 2 * B], F32, tag="psE", name="psE2")
            for m in range(2):
                sl = psE[:, m * B:(m + 1) * B]
                ms = slice(m * 128, (m + 1) * 128)
                mm(sl, w["tz2_k0"][:, ms], h_tz[:, 0:B], start=True, stop=False)
                mm(sl, w["tz2_k1"][:, ms], h_tz[:, B:], start=False, stop=True)
            o_m = wp.tile([L, B], F32, tag="o_m", name="o_m")
            nc.scalar.activation(o_m[:], psE[:, 0:B], AF.Identity,
                                 bias=w["tz2_bm"][:, 0:1])
            o_s = wp.tile([L, B], F32, tag="o_s", name="o_s")
            nc.scalar.activation(o_s[:], psE[:, B:], AF.Abs,
                                 bias=w["tz2_bs"][:, 0:1])
            nc.sync.dma_start(d_om[:], o_m[:])
            nc.sync.dma_start(d_os[:], o_s[:])

    nc.compile()
    return nc


# --------------------------------------------------------------------------
# host-side packing
# --------------------------------------------------------------------------
def _prep_in_maps(inputs, n_tp):
    F = np.float32
    d = {k: np.ascontiguousarray(np.asarray(v, F)) for k, v in inputs.items()}
    obs = d["obs_tps"][:n_tp]
    data = d["data"][:, :n_tp]

    dd = (obs[:-1] - obs[1:])[::-1]
    dts = np.concatenate([np.full((1,), -0.01, F), dd])
    dt_b = np.ascontiguousarray(np.broadcast_to(dts[None, :], (128, n_tp)))
    b2dt = np.ascontiguousarray(d["ode_b2"][:, None] * dts[None, :])

    # [t, c, subj], reversed in time, bf16
    x_rev = np.ascontiguousarray(data.transpose(1, 2, 0)[::-1]).astype(BF)

    ns_w1s = d["ns_w1"].copy()
    ns_w1s[:2 * L] *= F(0.5)

    def kx(w1, b1):
        return np.vstack([w1[2 * L:], b1[None, :]])

    bf = {
        "ug1_k0": d["ug_w1"][:L], "ug1_k1": d["ug_w1"][L:2 * L],
        "ug1_kx": kx(d["ug_w1"], d["ug_b1"]),
        "rg1_k0": d["rg_w1"][:L], "rg1_k1": d["rg_w1"][L:2 * L],
        "rg1_kx": kx(d["rg_w1"], d["rg_b1"]),
        "ns1_k0": ns_w1s[:L], "ns1_k1": ns_w1s[L:2 * L],
        "ns1_kx": kx(d["ns_w1"], d["ns_b1"]),
        "ode1_w": d["ode_w1"],
        "ode2_k0": d["ode_w2"][:128], "ode2_k1": d["ode_w2"][128:],
        "ug2_k0": d["ug_w2"][:128], "ug2_k1": d["ug_w2"][128:],
        "rg2_k0": d["rg_w2"][:128], "rg2_k1": d["rg_w2"][128:],
        "ns2_k0": d["ns_w2"][:128], "ns2_k1": d["ns_w2"][128:],
        "ns2_bm16": d["ns_b2"][None, :L],
        "neg_eye": -np.eye(L, dtype=F),
        "tz1_k0": d["tz_w1"][:L], "tz1_k1": d["tz_w1"][L:],
        "tz1_b": d["tz_b1"][None, :],
        "tz2_k0": d["tz_w2"][:128], "tz2_k1": d["tz_w2"][128:],
    }
    shared = {k: np.ascontiguousarray(v.astype(BF)) for k, v in bf.items()}
    shared["dt_b"] = dt_b
    shared["b2dt"] = b2dt
    shared["ode1_bc"] = np.ascontiguousarray(d["ode_b1"].reshape(2, 128).T)
    shared["ug2_bc"] = np.ascontiguousarray(d["ug_b2"][:, None] * F(0.5))
    shared["rg2_bc"] = np.ascontiguousarray(d["rg_b2"][:, None] * F(0.5))
    shared["ns2_bm"] = np.ascontiguousarray(d["ns_b2"][:L, None])
    shared["ns2_bs"] = np.ascontiguousarray(d["ns_b2"][L:, None])
    shared["tz2_bm"] = np.ascontiguousarray(d["tz_b2"][:L, None])
    shared["tz2_bs"] = np.ascontiguousarray(d["tz_b2"][L:, None])

    in_maps = []
    for c in range(N_CORES):
        m = dict(shared)
        m["x_rev"] = np.ascontiguousarray(x_rev[:, :, c * B:(c + 1) * B])
        in_maps.append(m)
    return in_maps


def kernel(**inputs):
    from concourse.bass_utils import run_bass_kernel_spmd

    key = (N_TP, SP_ITERS)
    if key not in _CACHE:
        _CACHE[key] = _build(*key)
    nc = _CACHE[key]

    in_maps = _prep_in_maps(inputs, N_TP)
    res = run_bass_kernel_spmd(nc, in_maps, list(range(N_CORES)))
    outs = res.results

    mean = np.empty((1, N_SUBJ, L), np.float32)
    std = np.empty((1, N_SUBJ, L), np.float32)
    for c in range(N_CORES):
        mean[0, c * B:(c + 1) * B] = outs[c]["out_m"].T
        std[0, c * B:(c + 1) * B] = outs[c]["out_s"].T
    return mean, std

